# revision 10
# baseline (speedup 1.0000x reference)
"""Trainium2 Bass kernel for nn_GCN_45028437131774 (gnn_message_passing).

3-layer GCN (MMGCN-style) over N=100k nodes / E=2M edges, dim 64.

v3 design — GPSIMD ap_gather aggregation (the v2 dma_gather path was
bottlenecked by per-token DMA descriptor generation on the Pool engine:
~6ns/token x 3.5M padded tokens = 26ms of Q7 time):
  - Nodes sharded by destination across 8 cores (12500 each, padded to
    12544 with interior zero-pad columns at [6250,6272) and [12522,12544)
    so both partition-groups of the table have zero rows).
  - Per-layer node table lives in each core's SBUF as bf16 [128, 50176]:
    partition p holds feature p%64 of "group" p//64; global node n
    (core c, padded-local l) maps to group g=l//6272, free position
    pos = c*6272 + (l%6272).  Replicated via a DRAM AllGather of per-core
    [128, 6272] shards + one bulk DMA load per layer.
  - Aggregation: padded-CSR slot grids addressed by PAIR index
    (pos//2, parity r=pos%2).  One gpsimd.ap_gather per batch of 3072
    slots gathers d=2 bf16 (the pair) per slot per partition; Q7 cores
    0-3 walk the group-0 slot stream, cores 4-7 the group-1 stream
    (independent per-core index streams).  The slot grid (chunk-of-dsts
    x K slots) is shared by both groups (K = max over both), so a single
    DVE tensor_reduce [128, cols, K] (stride-2 view selecting parity r)
    accumulates both z-halves at once into Z bf16 [128, 12544]
    (z_lo on partitions 0-63 from group-0 sources, z_hi on 64-127).
  - Dense layer per 512-col chunk: ps1 = Wzz.T @ Z + Wx.T @ X (PSUM
    accumulation; Wzz stacks conv_w twice so z_lo+z_hi folds inside the
    contraction), leaky, ps2 = W2.T @ s1, bias + leaky -> X bf16.
  - x0: feat-major matmul projection + column-norm via ones-matmul
    (sum of squares), rsqrt, partition-broadcast matmul, one DVE mul.

kernel(**inputs) -> (mu, logvar), both [100000, 64] fp32.
"""

import os
import sys

import numpy as np

for _p in ("/opt/trn_rl_repo",):
    if _p not in sys.path and os.path.isdir(_p):
        sys.path.insert(0, _p)

import concourse.bacc as bacc
import concourse.bass as bass
import concourse.mybir as mybir
import concourse.tile as tile

F32 = mybir.dt.float32
BF16 = mybir.dt.bfloat16
I16 = mybir.dt.int16


class Cfg:
    def __init__(self, n_user=50000, n_item=50000, dim_feat=128, d=64,
                 e=2_000_000, ncores=8, tb=3072, num_layer=3, neg=0.01):
        self.n_user, self.n_item = n_user, n_item
        self.n = n_user + n_item
        self.dim_feat, self.d, self.e = dim_feat, d, e
        self.ncores = ncores
        self.num_layer, self.neg = num_layer, neg
        self.shard = self.n // ncores            # 12500 real
        self.half = 6272                         # padded half size
        self.cols = 2 * self.half                # 12544 padded columns
        self.reala = 6250                        # real cols per half
        self.pairs = self.ncores * self.half // 2  # 25088 ap_gather elems
        self.tb = tb                             # slots per ap_gather
        # chunk grid over padded columns (shared by aggregation + dense)
        self.chunks = []
        o = 0
        while o < self.cols:
            w = min(512, self.cols - o)
            self.chunks.append((o, w))
            o += w

    # padded-local position of real local index lr in [0, 12500)
    def lpad(self, lr):
        return lr + 22 * (lr >= self.reala)


# ---------------------------------------------------------------- host prep

class Prep:
    pass


def prep_edges(cfg: Cfg, edge_index: np.ndarray) -> Prep:
    src = edge_index[0].astype(np.int64)
    dst = edge_index[1].astype(np.int64)
    nch = len(cfg.chunks)

    # source-side addressing (same for every consumer core)
    cs = src // cfg.shard
    ls = src % cfg.shard
    l_s = cfg.lpad(ls)
    g_s = l_s // cfg.half
    pos = cs * cfg.half + (l_s % cfg.half)
    pair_s = pos // 2
    r_s = pos % 2

    cd = dst // cfg.shard
    col_d = cfg.lpad(dst % cfg.shard)
    ch_d = np.minimum(col_d // 512, nch - 1)

    # unified K over all cores and both source groups (one SPMD program)
    counts = np.zeros((cfg.ncores, 2, 2, cfg.cols), dtype=np.int64)
    np.add.at(counts, (cd, r_s, g_s, col_d), 1)
    kmax = counts.max(axis=(0, 2))                      # [2, cols]
    K = np.zeros((2, nch), dtype=np.int64)
    for ch, (o, w) in enumerate(cfg.chunks):
        K[0, ch] = max(int(kmax[0, o:o + w].max()), 1)  # force Z write
        K[1, ch] = int(kmax[1, o:o + w].max())

    # batch packing: blocks in (r, ch) order, split at col boundaries
    segs = []          # (batch, off, r, ch, colpos0, ncols, K)
    bidx, cur = 0, 0
    for r in range(2):
        for ch in range(nch):
            kk = int(K[r, ch])
            if kk == 0:
                continue
            cw = cfg.chunks[ch][1]
            colpos = 0
            while colpos < cw:
                space = cfg.tb - cur
                fit = space // kk
                if fit == 0:
                    bidx, cur = bidx + 1, 0
                    continue
                take = min(fit, cw - colpos)
                segs.append((bidx, cur, r, ch, colpos, take, kk))
                cur += take * kk
                colpos += take
                if cur == cfg.tb:
                    bidx, cur = bidx + 1, 0
    nb = bidx + (1 if cur > 0 else 0)
    nslots = nb * cfg.tb

    # slot base per (r, ch, colpos)
    base = np.full((2, nch, 512), -1, dtype=np.int64)
    for (b, off, r, ch, cp0, ncols, kk) in segs:
        base[r, ch, cp0:cp0 + ncols] = (b * cfg.tb + off
                                        + np.arange(ncols) * kk)

    by_batch = [[] for _ in range(nb)]
    for (b, off, r, ch, cp0, ncols, kk) in segs:
        by_batch[b].append((off, r, ch, cp0, ncols, kk))

    p = Prep()
    p.batch_segs = by_batch
    p.nbatch = nb
    p.nslots = nslots
    p.gidx = []
    ch_start = np.array([o for o, _w in cfg.chunks], dtype=np.int64)
    for c in range(cfg.ncores):
        sel = np.nonzero(cd == c)[0]
        r_, g_, col_, ch_, pr_ = (r_s[sel], g_s[sel], col_d[sel],
                                  ch_d[sel], pair_s[sel])
        # rank of each edge within its (g, r, col) group
        key = (g_ * 2 + r_) * cfg.cols + col_
        order = np.argsort(key, kind='stable')
        key_s = key[order]
        starts = np.r_[0, np.nonzero(np.diff(key_s))[0] + 1]
        grp_len = np.diff(np.r_[starts, len(key_s)])
        k_rank = np.arange(len(key_s)) - np.repeat(starts, grp_len)
        inv = np.empty_like(order)
        inv[order] = np.arange(len(order))
        k_rank = k_rank[inv]

        slot = base[r_, ch_, col_ - ch_start[ch_]] + k_rank
        assert (slot >= 0).all() and (slot < nslots).all()

        zp = c * (cfg.half // 2) + cfg.reala // 2 + 5   # a zero pad pair
        arr = np.full((2, nslots), zp, dtype=np.int16)
        arr[g_, slot] = pr_.astype(np.int16)

        wrap0 = np.ascontiguousarray(arr[0].reshape(-1, 16).T)
        wrap1 = np.ascontiguousarray(arr[1].reshape(-1, 16).T)
        gx = np.empty((128, nslots // 16), dtype=np.int16)
        gx[0:64] = np.tile(wrap0, (4, 1))
        gx[64:128] = np.tile(wrap1, (4, 1))
        p.gidx.append(gx)
    return p


def prep_nodes(cfg: Cfg, features, preference, mlp_w, mlp_b):
    raws = np.zeros((cfg.ncores, cfg.dim_feat, cfg.cols), dtype=np.float32)
    projs = np.zeros((cfg.ncores, cfg.dim_feat, cfg.d), dtype=np.float32)
    biases = np.zeros((cfg.ncores, cfg.d, 1), dtype=np.float32)
    rc = np.r_[0:cfg.reala, cfg.half:cfg.half + cfg.reala]  # real cols
    for c in range(cfg.ncores):
        lo, hi = c * cfg.shard, (c + 1) * cfg.shard
        if hi <= cfg.n_user:
            raws[c][:cfg.d, rc] = preference[lo:hi].T
            projs[c][:cfg.d, :] = np.eye(cfg.d, dtype=np.float32)
        elif lo >= cfg.n_user:
            raws[c][:, rc] = features[lo - cfg.n_user:hi - cfg.n_user].T
            projs[c] = mlp_w.T
            biases[c][:, 0] = mlp_b
        else:
            raise AssertionError("shard straddles user/item boundary")
    return raws, projs, biases


# ---------------------------------------------------------------- builder

def build_program(cfg: Cfg, p: Prep):
    nc = bacc.Bacc("TRN2", target_bir_lowering=False, debug=False)
    d, df = cfg.d, cfg.dim_feat
    nlay = cfg.num_layer
    nch = len(cfg.chunks)
    nb = p.nbatch
    segs = p.batch_segs

    raw_d = nc.dram_tensor("raw", [df, cfg.cols], F32, kind="ExternalInput")
    proj_d = nc.dram_tensor("proj", [df, d], F32, kind="ExternalInput")
    bias0_d = nc.dram_tensor("bias0", [d, 1], F32, kind="ExternalInput")
    gidx_d = nc.dram_tensor("gidx", [128, nb * cfg.tb // 16], I16,
                            kind="ExternalInput")
    wzz_d = [nc.dram_tensor(f"wzz{i}", [2 * d, 2 * d], F32,
                            kind="ExternalInput") for i in range(nlay + 2)]
    wx_d = [nc.dram_tensor(f"wx{i}", [d, 2 * d], F32, kind="ExternalInput")
            for i in range(nlay + 2)]
    b1_d = [nc.dram_tensor(f"b1p{i}", [2 * d, 1], F32, kind="ExternalInput")
            for i in range(nlay + 2)]
    w2_d = [nc.dram_tensor(f"w2c{i}", [2 * d, d], F32, kind="ExternalInput")
            for i in range(nlay + 2)]
    gb_d = [nc.dram_tensor(f"gbp{i}", [d, 1], F32, kind="ExternalInput")
            for i in range(nlay + 2)]
    mu_d = nc.dram_tensor("mu_fm", [d, cfg.cols], F32, kind="ExternalOutput")
    lv_d = nc.dram_tensor("lv_fm", [d, cfg.cols], F32, kind="ExternalOutput")

    xshard_d = [nc.dram_tensor(f"xshard{i}", [128, cfg.half], BF16)
                for i in range(2)]
    tabdram_d = [nc.dram_tensor(f"tabdram{i}", [cfg.ncores * 128, cfg.half],
                                BF16, addr_space="Shared") for i in range(2)]
    rg = [list(range(cfg.ncores))]

    ID = mybir.ActivationFunctionType.Identity
    SQ = mybir.ActivationFunctionType.Square
    SQRT = mybir.ActivationFunctionType.Sqrt
    MUL = mybir.AluOpType.mult
    MAX = mybir.AluOpType.max
    ADD = mybir.AluOpType.add
    AX = mybir.AxisListType.X

    with tile.TileContext(nc) as tc, \
            tc.tile_pool(name="const", bufs=1) as const, \
            tc.tile_pool(name="big", bufs=1) as big, \
            tc.tile_pool(name="bt", bufs=2) as btpool, \
            tc.tile_pool(name="idx", bufs=2) as ipool, \
            tc.tile_pool(name="tmp", bufs=2) as tpool, \
            tc.tile_pool(name="s1", bufs=3) as s1pool, \
            tc.tile_pool(name="x0", bufs=2) as x0pool, \
            tc.tile_pool(name="ot", bufs=2) as opool, \
            tc.tile_pool(name="sc", bufs=3) as scpool, \
            tc.tile_pool(name="psA", bufs=2,
                         space=bass.MemorySpace.PSUM) as psA, \
            tc.tile_pool(name="psB", bufs=2,
                         space=bass.MemorySpace.PSUM) as psB:

        from concourse import library_config
        nc.gpsimd.load_library(library_config.ap_gather)

        def load_const(dram, shape, dtype=F32):
            t = const.tile(shape, dtype, tag=dram.name, name=dram.name + "_s")
            nc.sync.dma_start(t[:], dram[:])
            return t

        def load_const_bf16(dram, shape):
            stage = s1pool.tile(shape, F32, tag="s1", name=dram.name + "_st")
            nc.sync.dma_start(stage[:], dram[:])
            t = const.tile(shape, BF16, tag=dram.name + "b",
                           name=dram.name + "_b")
            nc.scalar.copy(t[:], stage[:])
            return t

        proj_s = load_const(proj_d, [df, d])
        bias0_s = load_const(bias0_d, [d, 1])
        b1_s = [load_const(x, [2 * d, 1]) for x in b1_d]
        gb_s = [load_const(x, [d, 1]) for x in gb_d]
        wzz_s = [load_const_bf16(x, [2 * d, 2 * d]) for x in wzz_d]
        wx_s = [load_const_bf16(x, [d, 2 * d]) for x in wx_d]
        w2_s = [load_const_bf16(x, [2 * d, d]) for x in w2_d]
        ones64 = const.tile([d, 1], F32, tag="o64", name="o64")
        nc.vector.memset(ones64[:], 1.0)
        ones164 = const.tile([1, d], F32, tag="o164", name="o164")
        nc.vector.memset(ones164[:], 1.0)

        table = big.tile([128, cfg.ncores * cfg.half], BF16, tag="table",
                         name="table")
        Z = big.tile([128, cfg.cols], BF16, tag="Z", name="Z")
        X = big.tile([d, cfg.cols], BF16, tag="X", name="X")

        def leaky(ap):
            nc.vector.scalar_tensor_tensor(ap, ap, cfg.neg, ap, MUL, MAX)

        def push_table(layer):
            xs, tb_ = xshard_d[layer % 2], tabdram_d[layer % 2]
            nc.sync.dma_start(xs.ap()[0:d, :], X[:, 0:cfg.half])
            nc.sync.dma_start(xs.ap()[d:2 * d, :], X[:, cfg.half:cfg.cols])
            nc.gpsimd.collective_compute(
                "AllGather", mybir.AluOpType.bypass, replica_groups=rg,
                ins=[xs[:]], outs=[tb_[:]])

        def load_table(layer):
            tb_ = tabdram_d[layer % 2]
            nc.sync.dma_start(
                table[:].rearrange("p (c j) -> p c j", c=cfg.ncores),
                tb_.ap().rearrange("(c p) j -> p c j", c=cfg.ncores, p=128))

        def aggregate():
            # bf16 Z: DVE reduce still accumulates fp32 internally; the
            # single rounding at writeout (~4e-3 rel) is within budget.
            tab_view = table[:].rearrange("p (n two) -> p n two", two=2)
            for b in range(nb):
                idxt = ipool.tile([128, cfg.tb // 16], I16, tag="idxt",
                                  name="idxt")
                nc.sync.dma_start(
                    idxt[:],
                    gidx_d[:, b * cfg.tb // 16:(b + 1) * cfg.tb // 16])
                bt = btpool.tile([128, cfg.tb, 2], BF16, tag="bt", name="bt")
                nc.gpsimd.ap_gather(bt[:], tab_view, idxt[:], channels=128,
                                    num_elems=cfg.pairs, d=2,
                                    num_idxs=cfg.tb)
                with nc.allow_low_precision(
                        reason="bf16 Z writeout; DVE accumulates fp32"):
                    for (off, r, ch, cp0, ncols, kk) in segs[b]:
                        c0 = cfg.chunks[ch][0] + cp0
                        view = bt[:, off:off + ncols * kk, r].rearrange(
                            "p (c k) -> p c k", k=kk)
                        if r == 0:
                            nc.vector.tensor_reduce(Z[:, c0:c0 + ncols],
                                                    view, axis=AX, op=ADD)
                        else:
                            tmp = tpool.tile([128, 512], F32, tag="tmp",
                                             name="tmp")
                            nc.vector.tensor_reduce(tmp[:, :ncols], view,
                                                    axis=AX, op=ADD)
                            nc.vector.tensor_add(Z[:, c0:c0 + ncols],
                                                 Z[:, c0:c0 + ncols],
                                                 tmp[:, :ncols])

        def dense_chunk(li, ch, final, out_ap=None):
            o, cw = cfg.chunks[ch]
            sl = slice(o, o + cw)
            ps1 = psA.tile([2 * d, 512], F32, tag="ps1", name="ps1")
            nc.tensor.matmul(ps1[:, :cw], wzz_s[li][:], Z[:, sl],
                             start=True, stop=False)
            nc.tensor.matmul(ps1[:, :cw], wx_s[li][:], X[:, sl],
                             start=False, stop=True)
            s1 = s1pool.tile([2 * d, 512], BF16, tag="s1b", name="s1")
            nc.scalar.activation(s1[:, :cw], ps1[:, :cw], ID,
                                 bias=b1_s[li][:])
            leaky(s1[:, :cw])
            ps2 = psB.tile([d, 512], F32, tag="ps2", name="ps2")
            nc.tensor.matmul(ps2[:, :cw], w2_s[li][:], s1[:, :cw])
            if final:
                ot = opool.tile([d, 512], F32, tag="ot", name="ot")
                nc.scalar.activation(ot[:, :cw], ps2[:, :cw], ID,
                                     bias=gb_s[li][:])
                nc.sync.dma_start(out_ap, ot[:, :cw])
            else:
                nc.scalar.activation(X[:, sl], ps2[:, :cw], ID,
                                     bias=gb_s[li][:])
                leaky(X[:, sl])

        # ------------------------------------------------------- main flow
        # x0: feat-major projection + column L2-normalize
        for ch in range(nch):
            o, cw = cfg.chunks[ch]
            sl = slice(o, o + cw)
            rawt = s1pool.tile([df, 512], F32, tag="s1", name="rawt")
            nc.sync.dma_start(rawt[:, :cw], raw_d[:, sl])
            ps0 = psA.tile([d, 512], F32, tag="ps1", name="ps0")
            nc.tensor.matmul(ps0[:d, :cw], proj_s[:], rawt[:, :cw])
            x0f = x0pool.tile([d, 512], F32, tag="x0f", name="x0f")
            nc.scalar.activation(x0f[:, :cw], ps0[:d, :cw], ID,
                                 bias=bias0_s[:])
            sqt = x0pool.tile([d, 512], F32, tag="sqt", name="sqt")
            nc.scalar.activation(sqt[:, :cw], x0f[:, :cw], SQ)
            pss = psB.tile([1, 512], F32, tag="ps2", name="pss")
            nc.tensor.matmul(pss[:1, :cw], ones64[:], sqt[:, :cw])
            ss = scpool.tile([1, 512], F32, tag="sc", name="ss")
            nc.vector.tensor_scalar_max(ss[:, :cw], pss[:1, :cw], 1e-24)
            nr = scpool.tile([1, 512], F32, tag="sc", name="nr")
            nc.scalar.activation(nr[:, :cw], ss[:, :cw], SQRT)
            inv = scpool.tile([1, 512], F32, tag="sc", name="inv")
            nc.vector.reciprocal(inv[:, :cw], nr[:, :cw])
            psb = psA.tile([d, 512], F32, tag="psb", name="psb")
            nc.tensor.matmul(psb[:d, :cw], ones164[:], inv[:, :cw])
            nc.vector.tensor_mul(X[:, sl], x0f[:, :cw], psb[:d, :cw])
        push_table(0)

        for li in range(nlay):
            load_table(li)
            aggregate()
            for ch in range(nch):
                dense_chunk(li, ch, final=False)
            # zero the interior pad columns before pushing
            nc.vector.memset(X[:, cfg.reala:cfg.half], 0.0)
            nc.vector.memset(X[:, cfg.half + cfg.reala:cfg.cols], 0.0)
            push_table(li + 1)

        load_table(nlay)
        aggregate()
        for ch in range(nch):
            o, cw = cfg.chunks[ch]
            dense_chunk(nlay, ch, final=True, out_ap=mu_d[:, o:o + cw])
            dense_chunk(nlay + 1, ch, final=True, out_ap=lv_d[:, o:o + cw])

    nc.compile()
    return nc


# ---------------------------------------------------------------- kernel()

def make_in_map(cfg, p, raws, projs, biases, conv_w, lin_w, lin_b, g_w, g_b,
                core):
    d = cfg.d
    m = dict(raw=np.ascontiguousarray(raws[core]),
             proj=np.ascontiguousarray(projs[core]),
             bias0=np.ascontiguousarray(biases[core]),
             gidx=p.gidx[core])
    for i in range(cfg.num_layer + 2):
        wzz = np.zeros((2 * d, 2 * d), np.float32)
        wzz[:d, :d] = conv_w[i]
        wzz[d:, :d] = conv_w[i]
        m[f"wzz{i}"] = wzz
        wx = np.zeros((d, 2 * d), np.float32)
        wx[:, d:] = lin_w[i].T
        m[f"wx{i}"] = wx
        b1 = np.zeros((2 * d, 1), np.float32)
        b1[d:, 0] = lin_b[i]
        m[f"b1p{i}"] = b1
        m[f"w2c{i}"] = np.ascontiguousarray(g_w[i].T)
        m[f"gbp{i}"] = np.ascontiguousarray(g_b[i].reshape(-1, 1))
    return m


def kernel(features, edge_index, preference, mlp_w, mlp_b,
           conv_w, lin_w, lin_b, g_w, g_b, cfg: Cfg | None = None):
    cfg = cfg or Cfg()
    features = np.asarray(features, dtype=np.float32)
    edge_index = np.asarray(edge_index)
    preference = np.asarray(preference, dtype=np.float32)
    mlp_w = np.asarray(mlp_w, dtype=np.float32)
    mlp_b = np.asarray(mlp_b, dtype=np.float32)
    conv_w = [np.asarray(w, dtype=np.float32) for w in conv_w]
    lin_w = [np.asarray(w, dtype=np.float32) for w in lin_w]
    lin_b = [np.asarray(w, dtype=np.float32) for w in lin_b]
    g_w = [np.asarray(w, dtype=np.float32) for w in g_w]
    g_b = [np.asarray(w, dtype=np.float32) for w in g_b]

    p = prep_edges(cfg, edge_index)
    raws, projs, biases = prep_nodes(cfg, features, preference, mlp_w, mlp_b)
    nc = build_program(cfg, p)
    in_maps = [make_in_map(cfg, p, raws, projs, biases, conv_w, lin_w,
                           lin_b, g_w, g_b, c) for c in range(cfg.ncores)]

    from concourse.bass_utils import run_bass_kernel_spmd
    global LAST_RESULTS
    LAST_RESULTS = run_bass_kernel_spmd(nc, in_maps, list(range(cfg.ncores)))
    results = LAST_RESULTS.results

    rc = np.r_[0:cfg.reala, cfg.half:cfg.half + cfg.reala]
    mu = np.concatenate(
        [results[c]["mu_fm"][:, rc] for c in range(cfg.ncores)], axis=1).T
    lv = np.concatenate(
        [results[c]["lv_fm"][:, rc] for c in range(cfg.ncores)], axis=1).T
    return np.ascontiguousarray(mu), np.ascontiguousarray(lv)
